# revision 11
# baseline (speedup 1.0000x reference)
"""InterleavedHeadAttention Trainium2 kernel.

Sharding (8 cores): core c handles batch b = c//4 and 4 output heads
[4*(c%4), 4*(c%4)+4).  The alpha head-mixing einsum is folded into the
QKV projection weights on the host, so each core's projections only
produce its own heads' (h, p, d) slices.  The pseudo-head merge uses
(p, n) flat ordering internally (attention is permutation invariant;
the token-causal mask depends only on n), which makes every layout a
direct view of a matmul output.  collapse and the 1/den softmax
normalization are applied on-device; Wo is folded with collapse and
applied per-head, each core emitting a partial (S, HID) bf16 output
that the host sums in f32 (+bo).

All per-core inputs are packed into a single 1-D bf16 "blob" tensor:
per-exec dispatch overhead in the PJRT/axon path scales with the number
of bound buffers, so 17 inputs -> 1 input is a large wall-clock win.

Compute structure (v2):
- Scores for one (Jn, pk) key block land in a 2-bank PSUM macro-tile
  [128, 1024] = [pq0 | pq1]; one Exp activation covers both halves.
  Score matmuls run full 512-wide even in diagonal blocks (the masked
  columns hold real, small q.k values); the AV matmuls only consume the
  causally-allowed column range, and the 128-wide diagonal sub-block is
  tri-masked on the probs.
- Softmax denominators are accumulated via an extra all-ones row in the
  V tiles (vaug); per query window the 8 (h, pq) denominator rows are
  copied into one [8, 512] tile, inverted with a single batched
  reciprocal, and broadcast back to 64 rows with a selector matmul.
- Loop order is window-outer (In), head-inner, so the output projection
  for window 0 overlaps the attention of window 1.
- Q/K biases are folded into the PSUM->SBUF eviction as per-partition
  tensor_scalar adds; V bias keeps the rank-1 ones-row matmul.
"""
import numpy as np
import ml_dtypes

import concourse.bacc as bacc
import concourse.bass as bass
import concourse.tile as tile
import concourse.mybir as mybir
from concourse.bass_utils import run_bass_kernel_spmd

B, S, HID, H, P = 2, 1024, 1024, 16, 2
D = HID // H          # 64
HL = 4                # heads per core
G = HL * P            # (h,p) groups per core = 8
HPD = HL * P * D      # 512 projection rows per core
BF = mybir.dt.bfloat16
F32 = mybir.dt.float32
NCORES = 8
KT = HID // 128       # 8 k tiles over hidden
NT = S // 512         # 2 query windows

# blob layout (bf16 element offsets)
OFF_XT = 0                            # (HID, S)
OFF_WQ = OFF_XT + HID * S             # (HID, HPD) each
OFF_WK = OFF_WQ + HID * HPD
OFF_WV = OFF_WK + HID * HPD
OFF_BQK = OFF_WV + HID * HPD          # (128, 12): bqT | bkT | bk2T
OFF_BV = OFF_BQK + 128 * 12           # (512,)
OFF_WO = OFF_BV + HPD                 # (HL, P*D, HID)
OFF_TRI = OFF_WO + HL * P * D * HID   # (128, 128)
TOT = OFF_TRI + 128 * 128

_compiled = None


def _build():
    nc = bacc.Bacc()
    blob = nc.dram_tensor("blob", (TOT,), BF, kind="ExternalInput")
    out = nc.dram_tensor("o", (S, HID), BF, kind="ExternalOutput")

    def bview(off, p, e):
        return blob[off:off + p * e].rearrange("(p e) -> p e", e=e)

    with tile.TileContext(nc) as tc:
        with tc.tile_pool(name="persist", bufs=1) as pp, \
             tc.tile_pool(name="ppool", bufs=6) as ppl, \
             tc.tile_pool(name="small", bufs=4) as sml, \
             tc.tile_pool(name="osb", bufs=3) as osb, \
             tc.tile_pool(name="psc", bufs=3, space=bass.MemorySpace.PSUM) as psc, \
             tc.tile_pool(name="psav", bufs=2, space=bass.MemorySpace.PSUM) as psav:

            ones = pp.tile([1, 512], BF, tag="ones", name="ones")
            nc.gpsimd.memset(ones[:], 1.0)
            ones4 = pp.tile([128, 64], BF, tag="ones4", name="ones4")
            nc.gpsimd.memset(ones4[:], 1.0)
            tri_sb = pp.tile([128, 128], BF, tag="tri", name="tri")
            nc.sync.dma_start(tri_sb[:], bview(OFF_TRI, 128, 128))
            bqk_bf = pp.tile([128, 12], BF, tag="bqkb", name="bqkb")
            nc.sync.dma_start(bqk_bf[:], bview(OFF_BQK, 128, 12))
            bqk_sb = pp.tile([128, 12], F32, tag="bqk", name="bqk")
            nc.vector.tensor_copy(bqk_sb[:], bqk_bf[:])
            bv_sb = pp.tile([1, HPD], BF, tag="bv", name="bv")
            nc.sync.dma_start(bv_sb[:], bview(OFF_BV, 1, HPD))

            xt_sb = [pp.tile([128, S], BF, tag=f"xt{k}", name=f"xt{k}") for k in range(KT)]
            for k in range(KT):
                nc.sync.dma_start(xt_sb[k][:], bview(OFF_XT + k * 128 * S, 128, S))
            w_sb = {}
            for nm, off in (("q", OFF_WQ), ("k", OFF_WK), ("v", OFF_WV)):
                w_sb[nm] = [pp.tile([128, HPD], BF, tag=f"w{nm}{k}", name=f"w{nm}{k}") for k in range(KT)]
                for k in range(KT):
                    nc.sync.dma_start(w_sb[nm][k][:],
                                      bview(off + k * 128 * HPD, 128, HPD))
            woe_sb = [pp.tile([128, HID], BF, tag=f"woe{h}", name=f"woe{h}") for h in range(HL)]
            for h in range(HL):
                nc.sync.dma_start(woe_sb[h][:],
                                  bview(OFF_WO + h * P * D * HID, 128, HID))

            # ---- Q/K transposed projections: out (hpd=512, n=1024) ----
            qt_sb = [pp.tile([128, S], BF, tag=f"qt{h}", name=f"qt{h}") for h in range(HL)]
            kt_sb = [pp.tile([128, S], BF, tag=f"kt{h}", name=f"kt{h}") for h in range(HL)]
            kt2_sb = [pp.tile([128, S], BF, tag=f"kt2{h}", name=f"kt2{h}") for h in range(HL)]
            for nm in ("q", "k"):
                bcol = 0 if nm == "q" else 4
                for mt in range(HL):          # 128 hpd rows = head mt
                    for nt in range(NT):      # 512 seq cols
                        acc = psc.tile([128, 512], F32, tag="sc", name="acc")
                        for k in range(KT):
                            nc.tensor.matmul(
                                acc[:], w_sb[nm][k][:, mt * 128:(mt + 1) * 128],
                                xt_sb[k][:, nt * 512:(nt + 1) * 512],
                                start=(k == 0), stop=(k == KT - 1))
                        dst = qt_sb[mt] if nm == "q" else kt_sb[mt]
                        sl = slice(nt * 512, (nt + 1) * 512)
                        nc.vector.tensor_scalar_add(
                            dst[:, sl], acc[:], bqk_sb[:, bcol + mt:bcol + mt + 1])
                        if nm == "k":
                            nc.vector.tensor_scalar_add(
                                kt2_sb[mt][0:64, sl], acc[64:128, :],
                                bqk_sb[0:64, 8 + mt:8 + mt + 1])
                            nc.vector.tensor_scalar_add(
                                kt2_sb[mt][64:128, sl], acc[0:64, :],
                                bqk_sb[64:128, 8 + mt:8 + mt + 1])

            # ---- V projection: out (n=1024, hpd=512) -> vaug (128, 8*65) ----
            vaug = [pp.tile([128, G * 65], BF, tag=f"va{j}", name=f"va{j}") for j in range(S // 128)]
            for jt in range(S // 128):
                v3 = vaug[jt].rearrange("p (g e) -> p g e", e=65)
                nc.gpsimd.memset(v3[:, :, 64:65], 1.0)
                acc = psc.tile([128, 512], F32, tag="sc", name="acc")
                for k in range(KT):
                    nc.tensor.matmul(
                        acc[:], xt_sb[k][:, jt * 128:(jt + 1) * 128],
                        w_sb["v"][k][:], start=(k == 0), stop=False)
                nc.tensor.matmul(acc[:], ones[:, 0:128], bv_sb[:],
                                 start=False, stop=True)
                nc.vector.tensor_copy(
                    v3[:, :, 0:64], acc[:].rearrange("p (g e) -> p g e", e=64))

            # ---- attention, window-outer / head-inner ----
            ot2 = [pp.tile([128, S], BF, tag=f"ot2{h}", name=f"ot2{h}") for h in range(HL)]
            oav = [[pp.tile([64, 512], BF, tag=f"oav{In}_{i}", name=f"oav{In}_{i}")
                    for i in range(G)] for In in range(NT)]
            # denominator rows live at partitions {0,32,64,96} of two tiles
            # (DVE writes must be 32-partition aligned); memset so the unused
            # lanes reciprocate 1.0 rather than garbage.
            den = [[pp.tile([128, 512], BF, tag=f"den{In}_{t}", name=f"den{In}_{t}")
                    for t in range(2)] for In in range(NT)]
            for In in range(NT):
                for t in range(2):
                    nc.gpsimd.memset(den[In][t][:], 1.0)
            for In in range(NT):
                JMAX = 4 * In + 4
                for h in range(HL):
                    avp = [psav.tile([65, 512], F32, tag="av", name="av") for _ in range(2)]
                    for Jn in range(JMAX):
                        diag = Jn >= 4 * In
                        c0 = 128 * (Jn - 4 * In) if diag else 0
                        jsl = slice(Jn * 128, (Jn + 1) * 128)
                        isl = slice(In * 512, (In + 1) * 512)
                        for pk in range(2):
                            lhsA = (kt_sb[h] if pk == 0 else kt2_sb[h])
                            lhsB = (kt2_sb[h] if pk == 0 else kt_sb[h])
                            mac = psc.tile([128, 1024], F32, tag="sc", name="mac")
                            nc.tensor.matmul(
                                mac[:, 0:512], lhsA[0:64, jsl],
                                qt_sb[h][0:64, isl], start=True, stop=True)
                            nc.tensor.matmul(
                                mac[:, 512:1024], lhsB[64:128, jsl],
                                qt_sb[h][64:128, isl], start=True, stop=True)
                            pt = ppl.tile([128, 1024], BF, tag="p", name="p")
                            nc.scalar.activation(
                                pt[:], mac[:],
                                mybir.ActivationFunctionType.Exp, scale=0.125)
                            if diag:
                                nc.vector.tensor_mul(
                                    pt[:, c0:c0 + 128], pt[:, c0:c0 + 128], tri_sb[:])
                                nc.vector.tensor_mul(
                                    pt[:, 512 + c0:512 + c0 + 128],
                                    pt[:, 512 + c0:512 + c0 + 128], tri_sb[:])
                            g = h * 2 + pk
                            first = (Jn == 0 and pk == 0)
                            last = (Jn == JMAX - 1 and pk == 1)
                            nc.tensor.matmul(
                                avp[0][:, c0:512],
                                vaug[Jn][:, g * 65:g * 65 + 65],
                                pt[:, c0:512], start=first, stop=last)
                            nc.tensor.matmul(
                                avp[1][:, c0:512],
                                vaug[Jn][:, g * 65:g * 65 + 65],
                                pt[:, 512 + c0:1024], start=first, stop=last)
                    for pq in range(2):
                        idx = h * 2 + pq
                        row = 32 * (idx % 4)
                        nc.vector.tensor_copy(
                            den[In][idx // 4][row:row + 1, :], avp[pq][64:65, :])
                        nc.vector.tensor_copy(oav[In][idx][:], avp[pq][0:64, :])
                # window tail: batched reciprocal, row-broadcast, normalize
                rec = [sml.tile([128, 512], BF, tag="rec", name="rec")
                       for _ in range(2)]
                with nc.allow_low_precision(reason="softmax recip bf16"):
                    nc.vector.reciprocal(rec[0][:], den[In][0][:])
                    nc.vector.reciprocal(rec[1][:], den[In][1][:])
                for h in range(HL):
                    for pq in range(2):
                        idx = h * 2 + pq
                        row = 32 * (idx % 4)
                        bcp = psav.tile([64, 512], F32, tag="av", name="bcp")
                        nc.tensor.matmul(
                            bcp[:], ones4[row:row + 1, :],
                            rec[idx // 4][row:row + 1, :], start=True, stop=True,
                            tile_position=(row, 0))
                        nc.vector.tensor_mul(
                            ot2[h][pq * 64:(pq + 1) * 64, In * 512:(In + 1) * 512],
                            oav[In][idx][:], bcp[:])
                # output projection for this window's seq blocks
                for mt in range(4 * In, 4 * In + 4):
                    for jt in range(HID // 512):
                        op = psc.tile([128, 512], F32, tag="sc", name="op")
                        for h in range(HL):
                            nc.tensor.matmul(
                                op[:], ot2[h][:, mt * 128:(mt + 1) * 128],
                                woe_sb[h][:, jt * 512:(jt + 1) * 512],
                                start=(h == 0), stop=(h == HL - 1))
                        ob = osb.tile([128, 512], BF, tag="ob", name="ob")
                        nc.vector.tensor_copy(ob[:], op[:])
                        nc.sync.dma_start(
                            out[mt * 128:(mt + 1) * 128, jt * 512:(jt + 1) * 512],
                            ob[:])
    nc.compile()
    return nc


def _prep(inputs):
    bf = ml_dtypes.bfloat16
    hs = np.asarray(inputs["hidden_states"], np.float32)
    maps = []
    tri = np.triu(np.ones((128, 128), np.float32)).astype(bf)  # tri[r,c]=1 iff c>=r
    eff = {}
    for nm in ("q", "k", "v"):
        W = np.asarray(inputs[f"W{nm}"], np.float32)
        bb = np.asarray(inputs[f"b{nm}"], np.float32)
        al = np.asarray(inputs[f"alpha_{nm}"], np.float32)
        We = np.einsum("mhp,mdc->hpdc", al, W.reshape(H, D, HID))
        be = np.einsum("mhp,md->hpd", al, bb.reshape(H, D))
        eff[nm] = (We, be)
    Wo = np.asarray(inputs["Wo"], np.float32)
    col = np.asarray(inputs["collapse"], np.float32)
    Woe = np.einsum("hp,jhd->hpdj", col, Wo.reshape(HID, H, D))  # (H,P,D,HID)
    for c in range(NCORES):
        b, g = c // 4, c % 4
        hs_sl = slice(g * HL, (g + 1) * HL)
        parts = [np.ascontiguousarray(hs[b].T).astype(bf).reshape(-1)]
        for nm in ("q", "k", "v"):
            We, _ = eff[nm]
            Wslice = We[hs_sl].reshape(HPD, HID)      # (hpd, c)
            parts.append(np.ascontiguousarray(Wslice.T).astype(bf).reshape(-1))
        # (128, 12) per-partition bias columns: bqT | bkT | bk2T
        bq = eff["q"][1][hs_sl].reshape(HL, 128).T    # (128, HL)
        bk = eff["k"][1][hs_sl].reshape(HL, 128).T
        bk2 = np.concatenate([bk[64:128], bk[0:64]], axis=0)
        parts.append(np.concatenate([bq, bk, bk2], axis=1).astype(bf).reshape(-1))
        parts.append(eff["v"][1][hs_sl].reshape(-1).astype(bf))
        parts.append(Woe[hs_sl].reshape(-1).astype(bf))
        parts.append(tri.reshape(-1))
        blob = np.concatenate(parts)
        assert blob.shape[0] == TOT, blob.shape
        maps.append({"blob": blob})
    return maps


def kernel(**inputs):
    global _compiled
    if _compiled is None:
        _compiled = _build()
    maps = _prep(inputs)
    res = run_bass_kernel_spmd(_compiled, maps, core_ids=list(range(NCORES)))
    bo = np.asarray(inputs["bo"], np.float32)
    out = np.zeros((B, S, HID), np.float32)
    for c in range(NCORES):
        out[c // 4] += res.results[c]["o"].astype(np.float32)
    out += bo
    return out
